# revision 29
# baseline (speedup 1.0000x reference)
"""Trainium2 Bass kernel for batched attention with query-axis softmax.

Reference computation (per example b of 64):
    Q = q @ Wq.T + bq              # [S=1024, Y=128]
    K = q @ Wk.T + bk
    V = q @ Wv.T + bv
    scores = Q @ K.T / sqrt(Y)     # [Sq, Sk]
    attn   = softmax(scores, axis=-2)   # normalize over the QUERY axis
    out    = attn @ V              # [S, Y]
    result = max(out, axis=-2)     # [Y]

Key structural facts exploited here:
  * softmax normalizes over q, which is NOT the contraction axis of attn@V:
    out[q,d] = sum_k U[q,k]/c[k] * V[k,d] with U = exp(scores),
    c[k] = sum_q U[q,k].  So the normalization folds into V's rows:
    out = U @ (V / c).  No SxS division needed.
  * storing scores transposed (scoresT[k,q]) makes c a free-dim row-sum,
    which the ScalarE Exp instruction produces for free via accum_out.
  * outT[d,q] = V'.T-accumulated matmul keeps the final max a free-dim
    reduce_max -> [128,1] per example.
  * V's bias is an extra K=1 matmul (ones x bv_row) accumulated into the
    same PSUM group -- free on PE, no free-dim broadcast op needed.

All matmul operands are bf16 (inputs rounded on host); accumulation is
always fp32 in PSUM and the softmax sums/normalization are fp32.

Sharding: data-parallel over batch, 8 examples per NeuronCore x 8 cores.
"""

import numpy as np
from contextlib import ExitStack

import concourse.bass as bass
import concourse.bacc as bacc
import concourse.tile as tile
import concourse.mybir as mybir
import concourse.bass_utils as bass_utils
import ml_dtypes

F32 = mybir.dt.float32
BF16 = mybir.dt.float16  # 16-bit matmul dtype: fp16 (11-bit significand)

NCORES = 8
B_PER_CORE = 8
S = 1024          # sequence length
X = 256           # input dim
Y = 128           # head dim
P = 128           # partitions
NH = 2            # 512-column halves of S (psum bank limit)
NKT = S // P      # 8 k-tiles


def emit(ctx, tc, out_d, ins):
    nc = tc.nc
    AF = mybir.ActivationFunctionType
    AX = mybir.AxisListType

    qt_d, w_d, b_d, row_d = ins

    wpool = ctx.enter_context(tc.tile_pool(name="w", bufs=1))
    qtp = ctx.enter_context(tc.tile_pool(name="qtp", bufs=4))
    qkp = ctx.enter_context(tc.tile_pool(name="qk", bufs=2))
    up = ctx.enter_context(tc.tile_pool(name="u", bufs=11))
    vrp = ctx.enter_context(tc.tile_pool(name="vr", bufs=4))
    vsp = ctx.enter_context(tc.tile_pool(name="vs", bufs=11))
    crp = ctx.enter_context(tc.tile_pool(name="cr", bufs=12))
    resp = ctx.enter_context(tc.tile_pool(name="res", bufs=2))
    # PSUM budget (8 banks): scores 2x2 + proj 1 + attnV-accum 2 + V 1
    pmm = ctx.enter_context(tc.tile_pool(name="pmm", bufs=2, space="PSUM"))
    pprj = ctx.enter_context(tc.tile_pool(name="pprj", bufs=1, space="PSUM"))
    pout = ctx.enter_context(tc.tile_pool(name="pout", bufs=1, space="PSUM"))
    pvp = ctx.enter_context(tc.tile_pool(name="pv", bufs=1, space="PSUM"))

    # Constants.
    # w: [128, 3*256] bf16 -- wq | wk | wv, each [128, 2*Y] (x-chunk xb at
    #    columns xb*Y..), projection scale folded into wq.
    # b: [128, 2] f32 -- bq_scaled | bk (per-partition bias for drains)
    # row: [1, 256] bf16 -- ones | bv (K=1 bias-matmul operands)
    w = wpool.tile([P, 7 * Y], BF16)
    nc.gpsimd.dma_start(w[:], w_d[:])
    bqk = wpool.tile([P, 2], F32)
    nc.gpsimd.dma_start(bqk[:], b_d[:])
    row = wpool.tile([1, 2 * Y], BF16)
    nc.gpsimd.dma_start(row[:], row_d[:])
    wq = w[:, 0 * Y: 2 * Y]
    wk = w[:, 2 * Y: 4 * Y]
    wv = w[:, 4 * Y: 6 * Y]

    def load_qt(b):
        # qT[b] : [256, 1024] -> sbuf [128, 2*1024], x-chunk xb at cols xb*S..
        qt = qtp.tile([P, 2 * S], BF16, tag="qt")
        qv = qt_d[b].rearrange("(xb p) s -> p xb s", p=P)
        nc.sync.dma_start(qt[:].rearrange("p (xb s) -> p xb s", xb=2), qv)
        return qt

    def proj_half(qt, dst, w_sb, bcol, nh):
        # One 512-column half of a Q/K projection: ZT[y, s_half] = W.T @ qT
        pm = pprj.tile([P, 512], F32, tag="pj")
        for xb in range(2):
            nc.tensor.matmul(
                pm[:],
                lhsT=w_sb[:, xb * Y:(xb + 1) * Y],
                rhs=qt[:, xb * S + nh * 512: xb * S + nh * 512 + 512],
                start=(xb == 0),
                stop=(xb == 1),
            )
        # psum -> sbuf with per-partition bias
        nc.vector.tensor_scalar_add(
            dst[:, nh * 512:(nh + 1) * 512], pm[:], bqk[:, bcol:bcol + 1]
        )

    def proj(qt):
        # Full projection (prologue only) -- uses the scores pool, which is
        # idle during the head, so the four halves double-buffer.
        QT = qkp.tile([P, S], BF16, tag="QT")
        KT = qkp.tile([P, S], BF16, tag="KT")
        for w_sb, bcol, dst in ((wq, 0, QT), (wk, 1, KT)):
            pm = pmm.tile([P, S], F32, tag="mm")
            for nh in range(NH):
                for xb in range(2):
                    nc.tensor.matmul(
                        pm[:, nh * 512:(nh + 1) * 512],
                        lhsT=w_sb[:, xb * Y:(xb + 1) * Y],
                        rhs=qt[:, xb * S + nh * 512: xb * S + nh * 512 + 512],
                        start=(xb == 0),
                        stop=(xb == 1),
                    )
            nc.vector.tensor_scalar_add(dst[:], pm[:], bqk[:, bcol:bcol + 1])
        return QT, KT

    def front(qt, QT, KT, kt):
        """scores -> exp(+colsum) -> V -> V/c for one k-tile; returns (u, vs)."""
        # scoresT[k_tile, q] = KT_chunk.T @ QT   (contract d)
        ps = pmm.tile([P, S], F32, tag="mm")
        with tc.high_priority(offset=40):
            for nh in range(NH):
                nc.tensor.matmul(
                    ps[:, nh * 512:(nh + 1) * 512],
                    lhsT=KT[:, kt * P:(kt + 1) * P],
                    rhs=QT[:, nh * 512: nh * 512 + 512],
                    start=True,
                    stop=True,
                )

            # U = exp(scoresT), c[k] = sum_q U (free accumulation on ACT)
            u = up.tile([P, S], BF16, tag="u")
            c = crp.tile([P, 1], F32, tag="c")
            nc.scalar.activation(u[:], ps[:], AF.Exp, accum_out=c[:])

        # V k-tile directly in [k, d] layout: V[s_tile,:] =
        #   qT_chunk.T @ WvT (+ ones.T @ bv_row for the bias)
        pv = pvp.tile([P, P], F32, tag="pv")
        for xb in range(2):
            nc.tensor.matmul(
                pv[:],
                lhsT=qt[:, xb * S + kt * P: xb * S + (kt + 1) * P],
                rhs=wv[:, xb * Y:(xb + 1) * Y],
                start=(xb == 0),
                stop=(xb == 1),
            )
        # Drain V out of PSUM right away (frees the single pv bank without
        # waiting for c), adding the bv bias via partition-broadcast.
        vraw = vrp.tile([P, P], BF16, tag="vr")
        nc.vector.tensor_add(vraw[:], pv[:], w[:, 6 * Y:7 * Y])

        # V'[k, :] = V[k, :] / c[k]
        r = crp.tile([P, 1], F32, tag="r")
        nc.vector.reciprocal(r[:], c[:])
        vs = vsp.tile([P, P], BF16, tag="vs")
        nc.vector.tensor_scalar_mul(vs[:], vraw[:], r[:])
        return u, vs

    # Software-pipelined emission over a flat (b, kt) step stream.  The
    # attnV accumulation runs LAG steps behind the scores->exp front so the
    # in-order PE always has the next exp's scores queued ahead of
    # slack-tolerant work (keeps ACT, the bottleneck engine, saturated), and
    # example b+1's DMA + projections are emitted inside example b's k-loop.
    LAG = 5
    steps = [(b, kt) for b in range(B_PER_CORE) for kt in range(NKT)]
    state = {}       # b -> (qt, QT, KT)
    fifo = {}        # step index -> (b, kt, u, vs)
    po = None

    qt0 = qtp.tile([P, 2 * S], BF16, tag="qt")
    qv0 = qt_d[0].rearrange("(xb p) s -> xb p s", p=P)
    for xb in range(2):
        nc.sync.dma_start(qt0[:, xb * S:(xb + 1) * S], qv0[xb])
    state[0] = (qt0, *proj(qt0))

    def drain(i):
        nonlocal po
        b, kt, u, vs = fifo.pop(i)
        if kt == 0:
            po = pout.tile([P, S], F32, tag="out")
        # outT[d, q] += V'.T @ U   (contract k)
        for nh in range(NH):
            nc.tensor.matmul(
                po[:, nh * 512:(nh + 1) * 512],
                lhsT=vs[:],
                rhs=u[:, nh * 512: nh * 512 + 512],
                start=(kt == 0),
                stop=(kt == NKT - 1),
            )
        if kt == NKT - 1:
            res = resp.tile([P, 1], F32, tag="res")
            nc.vector.reduce_max(res[:], po[:], axis=AX.X)
            nc.sync.dma_start(out_d[b].unsqueeze(1), res[:])

    qtiles = {0: qt0}
    for bb in range(1, min(3, B_PER_CORE)):
        qtiles[bb] = load_qt(bb)

    for i, (b, kt) in enumerate(steps):
        qt, QT, KT = state[b]
        if kt == 0 and b + 1 < B_PER_CORE:
            state[b + 1] = (qtiles[b + 1],)
        if kt == 1 and b + 3 < B_PER_CORE:
            qtiles[b + 3] = load_qt(b + 3)
        if kt == 2 and b + 1 < B_PER_CORE:
            # allocate next example's projection outputs; halves fill in
            # one per step over kt=2..5
            QT_n = qkp.tile([P, S], BF16, tag="QT")
            KT_n = qkp.tile([P, S], BF16, tag="KT")
            state[b + 1] = (state[b + 1][0], QT_n, KT_n)
        if 2 <= kt <= 5 and b + 1 < B_PER_CORE:
            qt_n, QT_n, KT_n = state[b + 1]
            w_sb, bcol, dst = ((wq, 0, QT_n), (wk, 1, KT_n))[(kt - 2) // 2]
            proj_half(qt_n, dst, w_sb, bcol, (kt - 2) % 2)
        u, vs = front(qt, QT, KT, kt)
        fifo[i] = (b, kt, u, vs)
        target = i - LAG
        if b == B_PER_CORE - 1 and kt >= 4:
            target = i - LAG + (kt - 3)  # taper: catch up 2/step at the end
        while fifo and min(fifo) <= target:
            drain(min(fifo))
    for i in sorted(fifo):
        drain(i)


def build_program():
    nc = bacc.Bacc(
        "TRN2",
        target_bir_lowering=False,
        debug=False,
        enable_asserts=False,
    )
    qt = nc.dram_tensor("qt", [B_PER_CORE, X, S], BF16, kind="ExternalInput").ap()
    w = nc.dram_tensor("w", [P, 7 * Y], BF16, kind="ExternalInput").ap()
    b = nc.dram_tensor("b", [P, 2], F32, kind="ExternalInput").ap()
    row = nc.dram_tensor("row", [1, 2 * Y], BF16, kind="ExternalInput").ap()
    out = nc.dram_tensor("out", [B_PER_CORE, Y], F32, kind="ExternalOutput").ap()

    ins = (qt, w, b, row)
    with tile.TileContext(nc) as tc:
        with ExitStack() as ctx:
            emit(ctx, tc, out, ins)
    nc.compile()
    return nc


_NC_CACHE = None


def _get_program():
    global _NC_CACHE
    if _NC_CACHE is None:
        _NC_CACHE = build_program()
    return _NC_CACHE


def prep_inputs(q, Wq, bq, Wk, bk, Wv, bv):
    """Host-side marshalling: transpose q, pack weights, fold softmax scale."""
    q = np.asarray(q, dtype=np.float32)
    scale = np.float32(1.0 / np.sqrt(Y))
    bf = np.float16

    qT = np.ascontiguousarray(q.transpose(0, 2, 1)).astype(bf)  # [B, X, S]

    def pack(w):  # [Y, X] torch layout -> [128, 2*Y]: chunk xb at cols xb*Y..
        wt = np.asarray(w, dtype=np.float32).T  # [X, Y]
        return np.concatenate([wt[0:P], wt[P:2 * P]], axis=1)

    w_all = np.concatenate(
        [pack(Wq) * scale, pack(Wk), pack(Wv),
         np.tile(np.asarray(bv, np.float32).reshape(1, Y), (P, 1))], axis=1
    ).astype(bf)
    b_all = np.stack(
        [np.asarray(bq, np.float32) * scale, np.asarray(bk, np.float32)], axis=1
    ).astype(np.float32)
    row = np.concatenate(
        [np.ones(Y, np.float32), np.asarray(bv, np.float32)]
    ).reshape(1, 2 * Y).astype(bf)

    feeds = {
        "w": np.ascontiguousarray(w_all),
        "b": np.ascontiguousarray(b_all),
        "row": np.ascontiguousarray(row),
    }
    return qT, feeds


def kernel(q, Wq, bq, Wk, bk, Wv, bv, _trace=False):
    qT, feeds = prep_inputs(q, Wq, bq, Wk, bk, Wv, bv)
    nc = _get_program()
    in_maps = [
        {"qt": qT[c * B_PER_CORE:(c + 1) * B_PER_CORE], **feeds}
        for c in range(NCORES)
    ]
    kw = {}
    if _trace:
        kw = dict(trace=True)
    res = bass_utils.run_bass_kernel_spmd(
        nc, in_maps, core_ids=list(range(NCORES)), **kw
    )
    out = np.concatenate([r["out"] for r in res.results], axis=0)
    if _trace:
        return out, res
    return out


# revision 32
# speedup vs baseline: 1.0066x; 1.0066x over previous
"""Trainium2 Bass kernel for batched attention with query-axis softmax.

Reference computation (per example b of 64):
    Q = q @ Wq.T + bq              # [S=1024, Y=128]
    K = q @ Wk.T + bk
    V = q @ Wv.T + bv
    scores = Q @ K.T / sqrt(Y)     # [Sq, Sk]
    attn   = softmax(scores, axis=-2)   # normalize over the QUERY axis
    out    = attn @ V              # [S, Y]
    result = max(out, axis=-2)     # [Y]

Key structural facts exploited here:
  * softmax normalizes over q, which is NOT the contraction axis of attn@V:
    out[q,d] = sum_k U[q,k]/c[k] * V[k,d] with U = exp(scores),
    c[k] = sum_q U[q,k].  So the normalization folds into V's rows:
    out = U @ (V / c).  No SxS division needed.
  * storing scores transposed (scoresT[k,q]) makes c a free-dim row-sum,
    which the ScalarE Exp instruction produces for free via accum_out.
  * outT[d,q] = V'.T-accumulated matmul keeps the final max a free-dim
    reduce_max -> [128,1] per example.
  * V's bias is an extra K=1 matmul (ones x bv_row) accumulated into the
    same PSUM group -- free on PE, no free-dim broadcast op needed.

All matmul operands are bf16 (inputs rounded on host); accumulation is
always fp32 in PSUM and the softmax sums/normalization are fp32.

Sharding: data-parallel over batch, 8 examples per NeuronCore x 8 cores.
"""

import numpy as np
from contextlib import ExitStack

import concourse.bass as bass
import concourse.bacc as bacc
import concourse.tile as tile
import concourse.mybir as mybir
import concourse.bass_utils as bass_utils
import ml_dtypes

F32 = mybir.dt.float32
BF16 = mybir.dt.float16  # 16-bit matmul dtype: fp16 (11-bit significand)

NCORES = 8
B_PER_CORE = 8
S = 1024          # sequence length
X = 256           # input dim
Y = 128           # head dim
P = 128           # partitions
NH = 2            # 512-column halves of S (psum bank limit)
NKT = S // P      # 8 k-tiles


def emit(ctx, tc, out_d, ins):
    nc = tc.nc
    AF = mybir.ActivationFunctionType
    AX = mybir.AxisListType

    qt_d, w_d, b_d, row_d = ins

    wpool = ctx.enter_context(tc.tile_pool(name="w", bufs=1))
    qtp = ctx.enter_context(tc.tile_pool(name="qtp", bufs=4))
    qkp = ctx.enter_context(tc.tile_pool(name="qk", bufs=2))
    up = ctx.enter_context(tc.tile_pool(name="u", bufs=11))
    vrp = ctx.enter_context(tc.tile_pool(name="vr", bufs=4))
    vsp = ctx.enter_context(tc.tile_pool(name="vs", bufs=11))
    crp = ctx.enter_context(tc.tile_pool(name="cr", bufs=12))
    resp = ctx.enter_context(tc.tile_pool(name="res", bufs=2))
    # PSUM budget (8 banks): scores 2x2 + proj 1 + attnV-accum 2 + V 1
    pmm = ctx.enter_context(tc.tile_pool(name="pmm", bufs=2, space="PSUM"))
    pprj = ctx.enter_context(tc.tile_pool(name="pprj", bufs=1, space="PSUM"))
    pout = ctx.enter_context(tc.tile_pool(name="pout", bufs=1, space="PSUM"))
    pvp = ctx.enter_context(tc.tile_pool(name="pv", bufs=1, space="PSUM"))

    # Constants.
    # w: [128, 3*256] bf16 -- wq | wk | wv, each [128, 2*Y] (x-chunk xb at
    #    columns xb*Y..), projection scale folded into wq.
    # b: [128, 2] f32 -- bq_scaled | bk (per-partition bias for drains)
    # row: [1, 256] bf16 -- ones | bv (K=1 bias-matmul operands)
    w = wpool.tile([P, 7 * Y], BF16)
    nc.gpsimd.dma_start(w[:], w_d[:])
    bqk = wpool.tile([P, 2], F32)
    nc.gpsimd.dma_start(bqk[:], b_d[:])
    row = wpool.tile([1, 2 * Y], BF16)
    nc.gpsimd.dma_start(row[:], row_d[:])
    wq = w[:, 0 * Y: 2 * Y]
    wk = w[:, 2 * Y: 4 * Y]
    wv = w[:, 4 * Y: 6 * Y]

    def load_qt(b):
        # qT[b] : [256, 1024] -> sbuf [128, 2*1024], x-chunk xb at cols xb*S..
        qt = qtp.tile([P, 2 * S], BF16, tag="qt")
        qv = qt_d[b].rearrange("(xb p) s -> p xb s", p=P)
        nc.sync.dma_start(qt[:].rearrange("p (xb s) -> p xb s", xb=2), qv)
        return qt

    def proj_half(qt, dst, w_sb, bcol, nh):
        # One 512-column half of a Q/K projection: ZT[y, s_half] = W.T @ qT
        pm = pprj.tile([P, 512], F32, tag="pj")
        for xb in range(2):
            nc.tensor.matmul(
                pm[:],
                lhsT=w_sb[:, xb * Y:(xb + 1) * Y],
                rhs=qt[:, xb * S + nh * 512: xb * S + nh * 512 + 512],
                start=(xb == 0),
                stop=(xb == 1),
            )
        # psum -> sbuf with per-partition bias
        nc.vector.tensor_scalar_add(
            dst[:, nh * 512:(nh + 1) * 512], pm[:], bqk[:, bcol:bcol + 1]
        )

    def proj(qt):
        # Full projection (prologue only) -- uses the scores pool, which is
        # idle during the head, so the four halves double-buffer.
        QT = qkp.tile([P, S], BF16, tag="QT")
        KT = qkp.tile([P, S], BF16, tag="KT")
        for w_sb, bcol, dst in ((wq, 0, QT), (wk, 1, KT)):
            pm = pmm.tile([P, S], F32, tag="mm")
            for nh in range(NH):
                for xb in range(2):
                    nc.tensor.matmul(
                        pm[:, nh * 512:(nh + 1) * 512],
                        lhsT=w_sb[:, xb * Y:(xb + 1) * Y],
                        rhs=qt[:, xb * S + nh * 512: xb * S + nh * 512 + 512],
                        start=(xb == 0),
                        stop=(xb == 1),
                    )
            nc.vector.tensor_scalar_add(dst[:], pm[:], bqk[:, bcol:bcol + 1])
        return QT, KT

    def front(qt, QT, KT, kt):
        """scores -> exp(+colsum) -> V -> V/c for one k-tile; returns (u, vs)."""
        # scoresT[k_tile, q] = KT_chunk.T @ QT   (contract d)
        ps = pmm.tile([P, S], F32, tag="mm")
        with tc.high_priority(offset=40):
            for nh in range(NH):
                nc.tensor.matmul(
                    ps[:, nh * 512:(nh + 1) * 512],
                    lhsT=KT[:, kt * P:(kt + 1) * P],
                    rhs=QT[:, nh * 512: nh * 512 + 512],
                    start=True,
                    stop=True,
                )

            # U = exp(scoresT), c[k] = sum_q U (free accumulation on ACT)
            u = up.tile([P, S], BF16, tag="u")
            c = crp.tile([P, 1], F32, tag="c")
            nc.scalar.activation(u[:], ps[:], AF.Exp, accum_out=c[:])

        # V k-tile directly in [k, d] layout: V[s_tile,:] =
        #   qT_chunk.T @ WvT (+ ones.T @ bv_row for the bias)
        pv = pvp.tile([P, P], F32, tag="pv")
        for xb in range(2):
            nc.tensor.matmul(
                pv[:],
                lhsT=qt[:, xb * S + kt * P: xb * S + (kt + 1) * P],
                rhs=wv[:, xb * Y:(xb + 1) * Y],
                start=(xb == 0),
                stop=(xb == 1),
            )
        # Drain V out of PSUM right away (frees the single pv bank without
        # waiting for c), adding the bv bias via partition-broadcast.
        vraw = vrp.tile([P, P], BF16, tag="vr")
        nc.vector.tensor_add(vraw[:], pv[:], w[:, 6 * Y:7 * Y])

        # V'[k, :] = V[k, :] / c[k]
        r = crp.tile([P, 1], F32, tag="r")
        nc.vector.reciprocal(r[:], c[:])
        vs = vsp.tile([P, P], BF16, tag="vs")
        nc.vector.tensor_scalar_mul(vs[:], vraw[:], r[:])
        return u, vs

    # Software-pipelined emission over a flat (b, kt) step stream.  The
    # attnV accumulation runs LAG steps behind the scores->exp front so the
    # in-order PE always has the next exp's scores queued ahead of
    # slack-tolerant work (keeps ACT, the bottleneck engine, saturated), and
    # example b+1's DMA + projections are emitted inside example b's k-loop.
    LAG = 5
    steps = [(b, kt) for b in range(B_PER_CORE) for kt in range(NKT)]
    state = {}       # b -> (qt, QT, KT)
    fifo = {}        # step index -> (b, kt, u, vs)
    po = None

    qt0 = qtp.tile([P, 2 * S], BF16, tag="qt")
    qv0 = qt_d[0].rearrange("(xb p) s -> xb p s", p=P)
    for xb in range(2):
        nc.sync.dma_start(qt0[:, xb * S:(xb + 1) * S], qv0[xb])
    state[0] = (qt0, *proj(qt0))

    def drain(i):
        nonlocal po
        b, kt, u, vs = fifo.pop(i)
        if kt == 0:
            po = pout.tile([P, S], F32, tag="out")
        # outT[d, q] += V'.T @ U   (contract k)
        for nh in range(NH):
            nc.tensor.matmul(
                po[:, nh * 512:(nh + 1) * 512],
                lhsT=vs[:],
                rhs=u[:, nh * 512: nh * 512 + 512],
                start=(kt == 0),
                stop=(kt == NKT - 1),
            )
        if kt == NKT - 1:
            res = resp.tile([P, 1], F32, tag="res")
            nc.vector.reduce_max(res[:], po[:], axis=AX.X)
            nc.sync.dma_start(out_d[b].unsqueeze(1), res[:])

    qtiles = {0: qt0}
    for bb in range(1, min(3, B_PER_CORE)):
        qtiles[bb] = load_qt(bb)

    for i, (b, kt) in enumerate(steps):
        qt, QT, KT = state[b]
        if kt == 0 and b + 1 < B_PER_CORE:
            state[b + 1] = (qtiles[b + 1],)
        if kt == 1 and b + 3 < B_PER_CORE:
            qtiles[b + 3] = load_qt(b + 3)
        if kt == 2 and b + 1 < B_PER_CORE:
            # allocate next example's projection outputs; halves fill in
            # one per step over kt=2..5
            QT_n = qkp.tile([P, S], BF16, tag="QT")
            KT_n = qkp.tile([P, S], BF16, tag="KT")
            state[b + 1] = (state[b + 1][0], QT_n, KT_n)
        if 2 <= kt <= 5 and b + 1 < B_PER_CORE:
            qt_n, QT_n, KT_n = state[b + 1]
            w_sb, bcol, dst = ((wq, 0, QT_n), (wk, 1, KT_n))[(kt - 2) // 2]
            proj_half(qt_n, dst, w_sb, bcol, (kt - 2) % 2)
        u, vs = front(qt, QT, KT, kt)
        fifo[i] = (b, kt, u, vs)
        target = i - LAG
        if b == B_PER_CORE - 1 and kt >= 4:
            target = i - LAG + (kt - 3)  # taper: catch up 2/step at the end
        while fifo and min(fifo) <= target:
            drain(min(fifo))
    for i in sorted(fifo):
        drain(i)


def build_program():
    nc = bacc.Bacc(
        "TRN2",
        target_bir_lowering=False,
        debug=False,
        enable_asserts=False,
    )
    qt = nc.dram_tensor("qt", [B_PER_CORE, X, S], BF16, kind="ExternalInput").ap()
    w = nc.dram_tensor("w", [P, 7 * Y], BF16, kind="ExternalInput").ap()
    b = nc.dram_tensor("b", [P, 2], F32, kind="ExternalInput").ap()
    row = nc.dram_tensor("row", [1, 2 * Y], BF16, kind="ExternalInput").ap()
    out = nc.dram_tensor("out", [B_PER_CORE, Y], F32, kind="ExternalOutput").ap()

    ins = (qt, w, b, row)
    with tile.TileContext(nc) as tc:
        with ExitStack() as ctx:
            emit(ctx, tc, out, ins)
    nc.compile()
    return nc


_NC_CACHE = None


def _get_program():
    global _NC_CACHE
    if _NC_CACHE is None:
        _NC_CACHE = build_program()
    return _NC_CACHE


def prep_inputs(q, Wq, bq, Wk, bk, Wv, bv):
    """Host-side marshalling: transpose q, pack weights, fold softmax scale."""
    q = np.asarray(q, dtype=np.float32)
    scale = np.float32(1.0 / np.sqrt(Y))
    bf = np.float16

    qT = np.ascontiguousarray(q.transpose(0, 2, 1)).astype(bf)  # [B, X, S]

    def pack(w):  # [Y, X] torch layout -> [128, 2*Y]: chunk xb at cols xb*Y..
        wt = np.asarray(w, dtype=np.float32).T  # [X, Y]
        return np.concatenate([wt[0:P], wt[P:2 * P]], axis=1)

    w_all = np.concatenate(
        [pack(Wq) * scale, pack(Wk), pack(Wv),
         np.tile(np.asarray(bv, np.float32).reshape(1, Y), (P, 1))], axis=1
    ).astype(bf)
    b_all = np.stack(
        [np.asarray(bq, np.float32) * scale, np.asarray(bk, np.float32)], axis=1
    ).astype(np.float32)
    row = np.concatenate(
        [np.ones(Y, np.float32), np.asarray(bv, np.float32)]
    ).reshape(1, 2 * Y).astype(bf)

    feeds = {
        "w": np.ascontiguousarray(w_all),
        "b": np.ascontiguousarray(b_all),
        "row": np.ascontiguousarray(row),
    }
    return qT, feeds


def kernel(q, Wq, bq, Wk, bk, Wv, bv, _trace=False):
    qT, feeds = prep_inputs(q, Wq, bq, Wk, bk, Wv, bv)
    nc = _get_program()
    in_maps = [
        {"qt": qT[c * B_PER_CORE:(c + 1) * B_PER_CORE], **feeds}
        for c in range(NCORES)
    ]
    kw = {}
    if _trace:
        kw = dict(trace=True)
    res = bass_utils.run_bass_kernel_spmd(
        nc, in_maps, core_ids=list(range(NCORES)), **kw
    )
    out = np.concatenate([r["out"] for r in res.results], axis=0)
    if _trace:
        return out, res
    return out


# revision 33
# speedup vs baseline: 1.0686x; 1.0616x over previous
"""Trainium2 Bass kernel for batched attention with query-axis softmax.

Reference computation (per example b of 64):
    Q = q @ Wq.T + bq              # [S=1024, Y=128]
    K = q @ Wk.T + bk
    V = q @ Wv.T + bv
    scores = Q @ K.T / sqrt(Y)     # [Sq, Sk]
    attn   = softmax(scores, axis=-2)   # normalize over the QUERY axis
    out    = attn @ V              # [S, Y]
    result = max(out, axis=-2)     # [Y]

Key structural facts exploited here:
  * softmax normalizes over q, which is NOT the contraction axis of attn@V:
    out[q,d] = sum_k U[q,k]/c[k] * V[k,d] with U = exp(scores),
    c[k] = sum_q U[q,k].  So the normalization folds into V's rows:
    out = U @ (V / c).  No SxS division needed.
  * storing scores transposed (scoresT[k,q]) makes c a free-dim row-sum,
    which the ScalarE Exp instruction produces for free via accum_out.
  * outT[d,q] = V'.T-accumulated matmul keeps the final max a free-dim
    reduce_max -> [128,1] per example.
  * V's bias is an extra K=1 matmul (ones x bv_row) accumulated into the
    same PSUM group -- free on PE, no free-dim broadcast op needed.

All matmul operands are bf16 (inputs rounded on host); accumulation is
always fp32 in PSUM and the softmax sums/normalization are fp32.

Sharding: data-parallel over batch, 8 examples per NeuronCore x 8 cores.
"""

import numpy as np
from contextlib import ExitStack

import concourse.bass as bass
import concourse.bacc as bacc
import concourse.tile as tile
import concourse.mybir as mybir
import concourse.bass_utils as bass_utils
import ml_dtypes

F32 = mybir.dt.float32
BF16 = mybir.dt.float16  # 16-bit matmul dtype: fp16 (11-bit significand)

NCORES = 8
B_PER_CORE = 8
S = 1024          # sequence length
X = 256           # input dim
Y = 128           # head dim
P = 128           # partitions
NH = 2            # 512-column halves of S (psum bank limit)
NKT = S // P      # 8 k-tiles


def emit(ctx, tc, out_d, ins):
    nc = tc.nc
    AF = mybir.ActivationFunctionType
    AX = mybir.AxisListType

    qt_d, w_d, b_d = ins

    wpool = ctx.enter_context(tc.tile_pool(name="w", bufs=1))
    qtp = ctx.enter_context(tc.tile_pool(name="qtp", bufs=4))
    qkp = ctx.enter_context(tc.tile_pool(name="qk", bufs=2))
    up = ctx.enter_context(tc.tile_pool(name="u", bufs=11))
    vrp = ctx.enter_context(tc.tile_pool(name="vr", bufs=4))
    vsp = ctx.enter_context(tc.tile_pool(name="vs", bufs=11))
    crp = ctx.enter_context(tc.tile_pool(name="cr", bufs=12))
    resp = ctx.enter_context(tc.tile_pool(name="res", bufs=1))
    # PSUM budget (8 banks): scores 2x2 + proj 1 + attnV-accum 2 + V 1
    pmm = ctx.enter_context(tc.tile_pool(name="pmm", bufs=2, space="PSUM"))
    pprj = ctx.enter_context(tc.tile_pool(name="pprj", bufs=1, space="PSUM"))
    pout = ctx.enter_context(tc.tile_pool(name="pout", bufs=1, space="PSUM"))
    pvp = ctx.enter_context(tc.tile_pool(name="pv", bufs=1, space="PSUM"))

    # Constants.
    # w: [128, 3*256] bf16 -- wq | wk | wv, each [128, 2*Y] (x-chunk xb at
    #    columns xb*Y..), projection scale folded into wq.
    # b: [128, 2] f32 -- bq_scaled | bk (per-partition bias for drains)
    # row: [1, 256] bf16 -- ones | bv (K=1 bias-matmul operands)
    w = wpool.tile([P, 7 * Y], BF16)
    nc.gpsimd.dma_start(w[:], w_d[:])
    bqk = wpool.tile([P, 2 + P], F32)
    nc.gpsimd.dma_start(bqk[:], b_d[:])
    wq = w[:, 0 * Y: 2 * Y]
    wk = w[:, 2 * Y: 4 * Y]
    wv = w[:, 4 * Y: 6 * Y]

    def load_qt(b):
        # qT[b] : [256, 1024] -> sbuf [128, 2*1024], x-chunk xb at cols xb*S..
        qt = qtp.tile([P, 2 * S], BF16, tag="qt")
        qv = qt_d[b].rearrange("(xb p) s -> p xb s", p=P)
        nc.sync.dma_start(qt[:].rearrange("p (xb s) -> p xb s", xb=2), qv)
        return qt

    def proj_half(qt, dst, w_sb, bcol, nh):
        # One 512-column half of a Q/K projection: ZT[y, s_half] = W.T @ qT
        pm = pprj.tile([P, 512], F32, tag="pj")
        for xb in range(2):
            nc.tensor.matmul(
                pm[:],
                lhsT=w_sb[:, xb * Y:(xb + 1) * Y],
                rhs=qt[:, xb * S + nh * 512: xb * S + nh * 512 + 512],
                start=(xb == 0),
                stop=(xb == 1),
            )
        # psum -> sbuf with per-partition bias
        nc.vector.tensor_scalar_add(
            dst[:, nh * 512:(nh + 1) * 512], pm[:], bqk[:, bcol:bcol + 1]
        )

    def proj(qt):
        # Full projection (prologue only) -- uses the scores pool, which is
        # idle during the head, so the four halves double-buffer.
        QT = qkp.tile([P, S], BF16, tag="QT")
        KT = qkp.tile([P, S], BF16, tag="KT")
        for w_sb, bcol, dst in ((wq, 0, QT), (wk, 1, KT)):
            pm = pmm.tile([P, S], F32, tag="mm")
            for nh in range(NH):
                for xb in range(2):
                    nc.tensor.matmul(
                        pm[:, nh * 512:(nh + 1) * 512],
                        lhsT=w_sb[:, xb * Y:(xb + 1) * Y],
                        rhs=qt[:, xb * S + nh * 512: xb * S + nh * 512 + 512],
                        start=(xb == 0),
                        stop=(xb == 1),
                    )
            nc.vector.tensor_scalar_add(dst[:], pm[:], bqk[:, bcol:bcol + 1])
        return QT, KT

    def front(qt, QT, KT, kt):
        """scores -> exp(+colsum) -> V -> V/c for one k-tile; returns (u, vs)."""
        # scoresT[k_tile, q] = KT_chunk.T @ QT   (contract d)
        ps = pmm.tile([P, S], F32, tag="mm")
        with tc.high_priority(offset=40):
            for nh in range(NH):
                nc.tensor.matmul(
                    ps[:, nh * 512:(nh + 1) * 512],
                    lhsT=KT[:, kt * P:(kt + 1) * P],
                    rhs=QT[:, nh * 512: nh * 512 + 512],
                    start=True,
                    stop=True,
                )

            # U = exp(scoresT), c[k] = sum_q U (free accumulation on ACT)
            u = up.tile([P, S], BF16, tag="u")
            c = crp.tile([P, 1], F32, tag="c")
            nc.scalar.activation(u[:], ps[:], AF.Exp, accum_out=c[:])

        # V k-tile directly in [k, d] layout: V[s_tile,:] =
        #   qT_chunk.T @ WvT (+ ones.T @ bv_row for the bias)
        pv = pvp.tile([P, P], F32, tag="pv")
        for xb in range(2):
            nc.tensor.matmul(
                pv[:],
                lhsT=qt[:, xb * S + kt * P: xb * S + (kt + 1) * P],
                rhs=wv[:, xb * Y:(xb + 1) * Y],
                start=(xb == 0),
                stop=(xb == 1),
            )
        # Drain V out of PSUM right away (frees the single pv bank without
        # waiting for c), adding the bv bias via partition-broadcast.
        vraw = vrp.tile([P, P], BF16, tag="vr")
        nc.vector.tensor_add(vraw[:], pv[:], w[:, 6 * Y:7 * Y])

        # V'[k, :] = V[k, :] / c[k]
        r = crp.tile([P, 1], F32, tag="r")
        nc.vector.reciprocal(r[:], c[:])
        vs = vsp.tile([P, P], BF16, tag="vs")
        nc.vector.tensor_scalar_mul(vs[:], vraw[:], r[:])
        return u, vs

    # Software-pipelined emission over a flat (b, kt) step stream.  The
    # attnV accumulation runs LAG steps behind the scores->exp front so the
    # in-order PE always has the next exp's scores queued ahead of
    # slack-tolerant work (keeps ACT, the bottleneck engine, saturated), and
    # example b+1's DMA + projections are emitted inside example b's k-loop.
    LAG = 5
    steps = [(b, kt) for b in range(B_PER_CORE) for kt in range(NKT)]
    state = {}       # b -> (qt, QT, KT)
    fifo = {}        # step index -> (b, kt, u, vs)
    po = None

    qt0 = qtp.tile([P, 2 * S], BF16, tag="qt")
    qv0 = qt_d[0].rearrange("(xb p) s -> xb p s", p=P)
    for xb in range(2):
        nc.sync.dma_start(qt0[:, xb * S:(xb + 1) * S], qv0[xb])
    state[0] = (qt0, *proj(qt0))

    def drain(i):
        nonlocal po
        b, kt, u, vs = fifo.pop(i)
        if kt == 0:
            po = pout.tile([P, S], F32, tag="out")
        # outT[d, q] += V'.T @ U   (contract k)
        for nh in range(NH):
            nc.tensor.matmul(
                po[:, nh * 512:(nh + 1) * 512],
                lhsT=vs[:],
                rhs=u[:, nh * 512: nh * 512 + 512],
                start=(kt == 0),
                stop=(kt == NKT - 1),
            )
        if kt == NKT - 1:
            nc.vector.reduce_max(res_all[:, b:b + 1], po[:], axis=AX.X)

    res_all = resp.tile([P, B_PER_CORE], F32, tag="res")

    qtiles = {0: qt0}
    for bb in range(1, min(3, B_PER_CORE)):
        qtiles[bb] = load_qt(bb)

    for i, (b, kt) in enumerate(steps):
        qt, QT, KT = state[b]
        if kt == 0 and b + 1 < B_PER_CORE:
            state[b + 1] = (qtiles[b + 1],)
        if kt == 1 and b + 3 < B_PER_CORE:
            qtiles[b + 3] = load_qt(b + 3)
        if kt == 2 and b + 1 < B_PER_CORE:
            # allocate next example's projection outputs; halves fill in
            # one per step over kt=2..5
            QT_n = qkp.tile([P, S], BF16, tag="QT")
            KT_n = qkp.tile([P, S], BF16, tag="KT")
            state[b + 1] = (state[b + 1][0], QT_n, KT_n)
        if 2 <= kt <= 5 and b + 1 < B_PER_CORE:
            qt_n, QT_n, KT_n = state[b + 1]
            w_sb, bcol, dst = ((wq, 0, QT_n), (wk, 1, KT_n))[(kt - 2) // 2]
            proj_half(qt_n, dst, w_sb, bcol, (kt - 2) % 2)
        u, vs = front(qt, QT, KT, kt)
        fifo[i] = (b, kt, u, vs)
        target = i - LAG
        if b == B_PER_CORE - 1 and kt >= 4:
            target = i - LAG + (kt - 3)  # taper: catch up 2/step at the end
        while fifo and min(fifo) <= target:
            drain(min(fifo))
    for i in sorted(fifo):
        drain(i)

    # Transpose the collected [128(d), 8(b)] results to [8, 128] on the PE
    # so the single output DMA is 8 dense 512B rows instead of 128 scattered
    # 4B descriptors (which hogs the DMA queue for ~7us).
    pt = pvp.tile([P, P], F32, tag="pv")
    nc.tensor.transpose(pt[0:B_PER_CORE, :], res_all[:], bqk[:, 2:2 + P])
    res_t = resp.tile([B_PER_CORE, P], F32, tag="rest")
    nc.vector.tensor_copy(res_t[:], pt[0:B_PER_CORE, :])
    nc.sync.dma_start(out_d[:], res_t[:])


def build_program():
    nc = bacc.Bacc(
        "TRN2",
        target_bir_lowering=False,
        debug=False,
        enable_asserts=False,
    )
    qt = nc.dram_tensor("qt", [B_PER_CORE, X, S], BF16, kind="ExternalInput").ap()
    w = nc.dram_tensor("w", [P, 7 * Y], BF16, kind="ExternalInput").ap()
    b = nc.dram_tensor("b", [P, 2 + P], F32, kind="ExternalInput").ap()
    out = nc.dram_tensor("out", [B_PER_CORE, Y], F32, kind="ExternalOutput").ap()

    ins = (qt, w, b)
    with tile.TileContext(nc) as tc:
        with ExitStack() as ctx:
            emit(ctx, tc, out, ins)
    nc.compile()
    return nc


_NC_CACHE = None


def _get_program():
    global _NC_CACHE
    if _NC_CACHE is None:
        _NC_CACHE = build_program()
    return _NC_CACHE


def prep_inputs(q, Wq, bq, Wk, bk, Wv, bv):
    """Host-side marshalling: transpose q, pack weights, fold softmax scale."""
    q = np.asarray(q, dtype=np.float32)
    scale = np.float32(1.0 / np.sqrt(Y))
    bf = np.float16

    qT = np.ascontiguousarray(q.transpose(0, 2, 1)).astype(bf)  # [B, X, S]

    def pack(w):  # [Y, X] torch layout -> [128, 2*Y]: chunk xb at cols xb*Y..
        wt = np.asarray(w, dtype=np.float32).T  # [X, Y]
        return np.concatenate([wt[0:P], wt[P:2 * P]], axis=1)

    w_all = np.concatenate(
        [pack(Wq) * scale, pack(Wk), pack(Wv),
         np.tile(np.asarray(bv, np.float32).reshape(1, Y), (P, 1))], axis=1
    ).astype(bf)
    b_all = np.concatenate(
        [np.stack([np.asarray(bq, np.float32) * scale,
                   np.asarray(bk, np.float32)], axis=1),
         np.eye(P, dtype=np.float32)], axis=1
    ).astype(np.float32)
    feeds = {
        "w": np.ascontiguousarray(w_all),
        "b": np.ascontiguousarray(b_all),
    }
    return qT, feeds


def kernel(q, Wq, bq, Wk, bk, Wv, bv, _trace=False):
    qT, feeds = prep_inputs(q, Wq, bq, Wk, bk, Wv, bv)
    nc = _get_program()
    in_maps = [
        {"qt": qT[c * B_PER_CORE:(c + 1) * B_PER_CORE], **feeds}
        for c in range(NCORES)
    ]
    kw = {}
    if _trace:
        kw = dict(trace=True)
    res = bass_utils.run_bass_kernel_spmd(
        nc, in_maps, core_ids=list(range(NCORES)), **kw
    )
    out = np.concatenate([r["out"] for r in res.results], axis=0)
    if _trace:
        return out, res
    return out
